# revision 26
# baseline (speedup 1.0000x reference)
"""ColorGNN Trainium2 kernel: 3-layer message passing on the complete bipartite
graph (50000 birds x 16 colors, H=128), sharded by birds across 8 NeuronCores.

Reformulation (validated vs reference to ~6e-3 rel in bf16):
  Split eW1[l] into W1a (bird-x part), W1b (color-x part), W1c (edge-attr part).
  Track h^l = relu(edge_in @ eW1[l] + eb1[l]) instead of edge_attr: then
    h^0      = relu(p * u0 + A0[i] + B0[c] + c0),   u0 = edge_W @ W1c0
    h^l      = relu(h^{l-1} @ Wf_l + A_l[i] + B_l[c] + c_l),  Wf_l = W2_{l-1} @ W1c_l
    aggr     = (sum h) @ W2_l + deg*eb2_l   (aggregation commutes with the linear W2)
  so the only per-edge matmuls are the layer transitions, and the scatter-adds
  become per-bird sums (local, 16 edges) + per-color sums (AllReduce of [H,16]).

Layout: per-tile h kept H-major [H=128 partitions, edges free], bird-major edge
order (e = i*16 + c), 112 birds per tile so the per-edge adds A[i]+B[c] are ONE
extra accumulating matmul with a constant two-hot rhs (rows 0..111 birds,
112..127 colors).
"""

import numpy as np
import ml_dtypes

import concourse.bass as bass
import concourse.mybir as mybir
import concourse.tile as tile
from concourse.bass_utils import run_bass_kernel_spmd

F32 = mybir.dt.float32
BF16 = mybir.dt.bfloat16
AF = mybir.ActivationFunctionType

NCORES = 8
N, C, H, L = 50000, 16, 128, 3
NB = N // NCORES            # 6250 birds per core
TB = 112                    # birds per tile
NT = (NB + TB - 1) // TB    # 56 tiles (last has 90 birds)
NE = TB * C                 # 1792 edge columns per full tile
HALF = 896                  # h psum half-tile (2 PSUM banks)


def _patch_tile_drain():
    """Kept for API compat; wait-splitting now happens in _split_multi_waits."""


def _split_multi_waits(nc):
    """walrus in this env allows only ONE sync-wait per instruction. For any
    instruction with more waits, hoist the extras onto same-engine nops
    inserted immediately before it (sequencers execute in program order)."""
    k = 0
    for f in nc.m.functions:
        for blk in f.blocks:
            insts = blk.instructions
            out = []
            for inst in insts:
                si = inst.sync_info
                if si is not None and si.on_wait and len(si.on_wait) > 1:
                    waits = list(si.on_wait)
                    for w in waits[:-1]:
                        nop = mybir.InstNoOp(
                            name=f"waitnop-{k}", engine=inst.engine
                        )
                        k += 1
                        nop.sync_info = mybir.SyncInfo(on_wait=[w], on_update=[])
                        out.append(nop)
                    si.on_wait = waits[-1:]
                out.append(inst)
            if len(out) != len(insts):
                blk.instructions = out


def _bf(a):
    return np.ascontiguousarray(np.asarray(a, np.float64)).astype(ml_dtypes.bfloat16)


def _f32(a):
    return np.ascontiguousarray(np.asarray(a, np.float64).astype(np.float32))


def _consts(inp):
    """Host-side weight folding. Returns dict of name -> np array (replicated)."""
    f = {k: np.asarray(v, np.float64) for k, v in inp.items()}
    eW1, eb1, eW2, eb2 = f["eW1"], f["eb1"], f["eW2"], f["eb2"]
    nW1, nb1, nW2, nb2 = f["nW1"], f["nb1"], f["nW2"], f["nb2"]
    W1a = [eW1[l][:H] for l in range(L)]
    W1b = [eW1[l][H : 2 * H] for l in range(L)]
    W1c = [eW1[l][2 * H :] for l in range(L)]
    W2 = [eW2[l] for l in range(L)]
    U1 = [nW1[l][:H] for l in range(L)]
    U2 = [nW1[l][H:] for l in range(L)]

    c = {}
    c["u0"] = _bf(f["edge_W"][0] @ W1c[0]).reshape(1, H)  # lhsT [1, H]
    cvec = [
        f["edge_b"] @ W1c[0] + eb1[0],
        eb2[0] @ W1c[1] + eb1[1],
        eb2[1] @ W1c[2] + eb1[2],
    ]
    wmats, vvecs = {}, {}
    for l in range(L):
        vvecs[f"cvec{l}"] = cvec[l]
        wmats[f"W1a{l}"] = W1a[l]               # rhs [H, H]
        wmats[f"W1b{l}"] = W1b[l]               # rhs [H, H]
        wmats[f"U1{l}"] = U1[l]                 # lhsT [H, H]
        wmats[f"W2U2{l}"] = W2[l] @ U2[l]       # lhsT [H, H]
        wmats[f"V{l}"] = nW2[l]                 # lhsT [H, H]
        vvecs[f"bb{l}"] = nb1[l] + C * (eb2[l] @ U2[l])
        vvecs[f"nb2{l}"] = nb2[l]
        if l < L - 1:
            vvecs[f"bc{l}"] = nb1[l] + N * (eb2[l] @ U2[l])
    wmats["Wf1"] = W2[0] @ W1c[1]               # lhsT [H, H]
    wmats["Wf2"] = W2[1] @ W1c[2]               # lhsT [H, H]
    vvecs["node_b"] = f["node_b"]
    # layer-0 folds: x0 = p @ node_W + node_b never materialized on device
    vvecs["cvec0"] = vvecs["cvec0"] + f["node_b"] @ W1a[0]
    vvecs["bb0"] = vvecs["bb0"] + f["node_b"] @ U1[0]
    c["NW1a"] = _bf(f["node_W"] @ W1a[0])       # [C, H]
    c["NWU1"] = _bf(f["node_W"] @ U1[0])        # [C, H]
    c["wpack"] = _bf(np.concatenate([wmats[k] for k in sorted(wmats)], axis=1))
    c["vpack"] = _f32(np.stack([vvecs[k] for k in sorted(vvecs)], axis=1))
    c["node_W"] = _bf(f["node_W"])              # lhsT [C, H]
    c["xc0T"] = _bf((f["node_W"] + f["node_b"]).T)  # [H, C]
    c["color_W"] = _bf(f["color_W"])            # lhsT [H, C]
    c["color_b"] = _f32(f["color_b"]).reshape(C, 1)
    # two-hot matrix for the A/B add: col e -> 1 at row e//16 and row 112+e%16
    oh = np.zeros((H, NE), np.float32)
    e = np.arange(NE)
    oh[e // C, e] = 1.0
    oh[TB + (e % C), e] = 1.0
    c["twohot"] = oh.astype(ml_dtypes.bfloat16)
    return c


def build_nc(sim_mode=False, skew=1, hin_bufs=3, hout_bufs=3, ab_bufs=3,
             hps_bufs=2, sps_bufs=4, l2_dve_relu=False):
    nc = bass.Bass(num_devices=1 if sim_mode else NCORES)

    wnames = sorted(
        [f"{nm}{l}" for l in range(L) for nm in ("W1a", "W1b", "U1", "W2U2", "V")]
        + ["Wf1", "Wf2"]
    )
    vnames = sorted(
        [f"cvec{l}" for l in range(L)] + [f"bb{l}" for l in range(L)]
        + [f"nb2{l}" for l in range(L)] + ["bc0", "bc1", "node_b"]
    )
    cshapes = {
        "u0": ([1, H], BF16), "node_W": ([C, H], BF16),
        "xc0T": ([H, C], BF16), "color_W": ([H, C], BF16), "color_b": ([C, 1], F32),
        "twohot": ([H, NE], BF16),
        "NW1a": ([C, H], BF16), "NWU1": ([C, H], BF16),
        "wpack": ([H, len(wnames) * H], BF16),
        "vpack": ([H, len(vnames)], F32),
    }

    dins = {}
    for name, (shp, dt) in cshapes.items():
        dins[name] = nc.declare_dram_parameter(name, shp, dt, isOutput=False)
    dins["pT"] = nc.declare_dram_parameter("pT", [C, NB], F32, isOutput=False)
    dins["pTbf"] = nc.declare_dram_parameter("pTbf", [C, NB], BF16, isOutput=False)
    dins["pbf"] = nc.declare_dram_parameter("pbf", [1, NB * C], BF16, isOutput=False)
    out_d = nc.declare_dram_parameter("outT", [C, NB], F32, isOutput=True)

    with tile.TileContext(nc) as tc:
        with (
            tc.tile_pool(name="const", bufs=1) as constp,
            tc.tile_pool(name="xpool", bufs=1) as xpool,
            tc.tile_pool(name="hsb", bufs=3) as hsbp,
            tc.tile_pool(name="ab", bufs=ab_bufs) as abp,
            tc.tile_pool(name="ptile", bufs=2) as ptp,
            tc.tile_pool(name="agg", bufs=4) as aggp,
            tc.tile_pool(name="small", bufs=4) as smallp,
            tc.tile_pool(name="hps", bufs=hps_bufs, space="PSUM") as hps,
            tc.tile_pool(name="sps", bufs=sps_bufs, space="PSUM") as sps,
            tc.tile_pool(name="dram", bufs=1, space="DRAM") as dramp,
        ):
            # ---- load constants ----
            cs = {}
            for name, (shp, dt) in cshapes.items():
                cs[name] = constp.tile(shp, dt, name=f"c_{name}")
                nc.sync.dma_start(out=cs[name][:], in_=dins[name][:])
            for i, nm in enumerate(wnames):
                cs[nm] = cs["wpack"][:, i * H : (i + 1) * H]
            for i, nm in enumerate(vnames):
                cs[nm] = cs["vpack"][:, i : i + 1]
            # p data resident in SBUF (cuts per-tile DMA count)
            pTbf_sb = constp.tile([C, NB], BF16, name="pTbf_sb")
            nc.sync.dma_start(out=pTbf_sb[:], in_=dins["pTbf"][:])
            pT_sb = constp.tile([C, NB], F32, name="pT_sb")
            nc.sync.dma_start(out=pT_sb[:], in_=dins["pT"][:])
            outT_sb = constp.tile([C, NB], F32, name="outT_sb")

            xT = xpool.tile([H, NT * TB], BF16)       # bird states, H-major
            xcT = xpool.tile([H, C], BF16)            # color states
            nc.sync.dma_start(out=xcT[:], in_=dins["xc0T"][:])
            csum_acc = xpool.tile([H, C], F32)
            csum_acc2 = xpool.tile([H, C], F32)

            H0R = 20   # h0 tiles resident in SBUF (no DRAM round-trip)
            h0_sb = constp.tile([H, H0R * NE], BF16, name="h0_sb")
            h_d = [dramp.tile([H, NT * NE], BF16, name=f"h_d{i}") for i in range(2)]
            AB_SLOTS = 4
            ab_tiles = [xpool.tile([H, H], BF16, name=f"abslot{i}")
                        for i in range(AB_SLOTS)]
            cc_in = [dramp.tile([H, C], F32, name=f"cc_in{i}") for i in range(2)]
            cc_out = [dramp.tile([H, C], F32, name=f"cc_out{i}") for i in range(2)]

            def tcols(t):
                nb = min(TB, NB - t * TB)
                return t * TB, nb

            # ================= pass l =================
            for l in range(L):
                last = l == L - 1
                # B_l [C, H] bird-major for the two-hot lhsT rows 112:128
                src = cs["xc0T"] if l == 0 else xcT
                b_ps = sps.tile([C, H], F32, name="smp", tag="sm")
                nc.tensor.matmul(b_ps[:], lhsT=src[:], rhs=cs[f"W1b{l}"][:],
                                 start=True, stop=True)
                B_sb = smallp.tile([C, H], BF16)
                nc.scalar.copy(B_sb[:], b_ps[:])
                for abt in ab_tiles:
                    nc.sync.dma_start(out=abt[TB:, :], in_=B_sb[:])
                if not last:
                    nc.vector.memset(csum_acc[:], 0.0)
                    nc.vector.memset(csum_acc2[:], 0.0)

                def stage_h(t):
                    """DMA in, build AB, h matmuls + relu into the pair buf."""
                    t0, nb = tcols(t)
                    ne = nb * C
                    ec0 = t * NE
                    half_off = (t % 2) * NE

                    prow = hin = None
                    if l == 0:
                        if t % 2 == 0:
                            pg_tiles[0] = ptp.tile([1, 2 * NE], BF16,
                                                   tag="prow", name="prow")
                            c0 = t * TB * C
                            cn = min(2 * NE, NB * C - c0)
                            nc.sync.dma_start(out=pg_tiles[0][:, :cn],
                                              in_=dins["pbf"][:, c0 : c0 + cn])
                        prow = pg_tiles[0][:, half_off : half_off + ne]
                    elif l == 1 and t < H0R:
                        hin = h0_sb[:, t * NE : t * NE + NE]
                    else:
                        if t % 2 == 0:
                            hp = hsbp.tile([H, 2 * NE], BF16, tag="hin",
                                           name="hin", bufs=hin_bufs)
                            cn = min(2 * NE, NB * C - ec0)
                            nc.sync.dma_start(out=hp[:, :cn],
                                              in_=h_d[l - 1][:, ec0 : ec0 + cn])
                            hin_pair[0] = hp
                        hin = hin_pair[0][:, half_off : half_off + ne]

                    # A_l = x^l @ W1a_l, bird-major [nb, H]
                    # (l=0: A0 = p @ (node_W @ W1a0), folded on host)
                    a_ps = sps.tile([TB, H], F32, name="smp", tag="sm")
                    if l == 0:
                        nc.tensor.matmul(a_ps[:nb, :],
                                         lhsT=pTbf_sb[:, t0 : t0 + nb],
                                         rhs=cs["NW1a"][:], start=True, stop=True)
                    else:
                        nc.tensor.matmul(a_ps[:nb, :], lhsT=xT[:, t0 : t0 + nb],
                                         rhs=cs[f"W1a{l}"][:], start=True, stop=True)
                    ab = ab_tiles[t % AB_SLOTS]
                    if nb < TB:
                        nc.vector.memset(ab[:TB, :], 0.0)
                    nc.vector.tensor_copy(ab[:nb, :], a_ps[:nb, :])

                    # ---- h tile: psum halves + relu into pair buffer ----
                    if l == 0 and t < H0R:
                        h_sb = h0_sb[:, t * NE : t * NE + NE]
                    else:
                        if t % 2 == 0:
                            hout_pair[0] = hsbp.tile([H, 2 * NE], BF16,
                                                     tag="hout", name="h_sb",
                                                     bufs=hout_bufs)
                        h_sb = hout_pair[0][:, half_off : half_off + ne]
                    for cbase in range(0, ne, HALF):
                        cw = min(HALF, ne - cbase)
                        ps = hps.tile([H, HALF], F32, name="hps")
                        for q0 in range(0, cw, 512):
                            qw = min(512, cw - q0)
                            sl = slice(cbase + q0, cbase + q0 + qw)
                            if l == 0:
                                nc.tensor.matmul(ps[:, q0 : q0 + qw],
                                                 lhsT=cs["u0"][:],
                                                 rhs=prow[:, sl],
                                                 start=True, stop=False)
                            else:
                                nc.tensor.matmul(ps[:, q0 : q0 + qw],
                                                 lhsT=cs[f"Wf{l}"][:],
                                                 rhs=hin[:, sl],
                                                 start=True, stop=False)
                            nc.tensor.matmul(ps[:, q0 : q0 + qw], lhsT=ab[:],
                                             rhs=cs["twohot"][:, sl],
                                             start=False, stop=True)
                        if l2_dve_relu and last and cbase > 0:
                            nc.vector.tensor_scalar(
                                h_sb[:, cbase : cbase + cw], ps[:, :cw],
                                cs[f"cvec{l}"][:], 0.0,
                                op0=mybir.AluOpType.add,
                                op1=mybir.AluOpType.max)
                        else:
                            nc.scalar.activation(h_sb[:, cbase : cbase + cw],
                                                 ps[:, :cw], AF.Relu,
                                                 bias=cs[f"cvec{l}"][:])

                    if l == 0 and t < H0R:
                        return h0_sb[:, (t & ~1) * NE :]
                    return hout_pair[0]

                def stage_node(te, h_pair):
                    """Aggregates + bird node update for tile pair (te, te+1).
                    The bird-sum over each bird's 16 edges rides the z matmul
                    group as 16 strided accumulating matmuls (fuses the W2@U2
                    projection of the aggregate for free)."""
                    t0, nb0 = tcols(te)
                    _, nb1 = tcols(te + 1)
                    nb = nb0 + nb1
                    ne = nb * C
                    ec0 = te * NE
                    h3 = h_pair[:, :ne].rearrange("p (b c) -> p b c", c=C)
                    if not last:
                        if not (l == 0 and te < H0R):
                            nc.sync.dma_start(out=h_d[l][:, ec0 : ec0 + ne],
                                              in_=h_pair[:, :ne])
                        csum_t = aggp.tile([H, C], F32, tag="csumt", name="csum_t")
                        nc.vector.tensor_reduce(csum_t[:], h3.transpose([0, 2, 1]),
                                                axis=mybir.AxisListType.X,
                                                op=mybir.AluOpType.add)
                        acc = csum_acc if (te // 2) % 2 == 0 else csum_acc2
                        nc.vector.tensor_add(acc[:], acc[:], csum_t[:])
                    z_ps = sps.tile([H, 2 * TB], F32, name="smp", tag="sm")
                    if l == 0:
                        nc.tensor.matmul(z_ps[:, :nb], lhsT=cs["NWU1"][:],
                                         rhs=pTbf_sb[:, t0 : t0 + nb],
                                         start=True, stop=False)
                    else:
                        nc.tensor.matmul(z_ps[:, :nb], lhsT=cs[f"U1{l}"][:],
                                         rhs=xT[:, t0 : t0 + nb],
                                         start=True, stop=False)
                    for cc in range(C):
                        nc.tensor.matmul(z_ps[:, :nb], lhsT=cs[f"W2U2{l}"][:],
                                         rhs=h3[:, :, cc], start=False,
                                         stop=(cc == C - 1))
                    s_sb = smallp.tile([H, 2 * TB], BF16, tag="ssb", name="s_sb")
                    nc.scalar.activation(s_sb[:, :nb], z_ps[:, :nb], AF.Relu,
                                         bias=cs[f"bb{l}"][:])
                    x_ps = sps.tile([H, 2 * TB], F32, name="smp", tag="sm")
                    nc.tensor.matmul(x_ps[:, :nb], lhsT=cs[f"V{l}"][:],
                                     rhs=s_sb[:, :nb], start=True, stop=True)
                    nc.scalar.activation(xT[:, t0 : t0 + nb], x_ps[:, :nb],
                                         AF.Identity, bias=cs[f"nb2{l}"][:])

                    if last:
                        # scores_T = color_W.T @ x3 + color_b ; out = scores * p
                        sc_ps = sps.tile([C, 2 * TB], F32, name="smp", tag="sm")
                        nc.tensor.matmul(sc_ps[:, :nb], lhsT=cs["color_W"][:],
                                         rhs=xT[:, t0 : t0 + nb],
                                         start=True, stop=True)
                        sc_sb = smallp.tile([C, 2 * TB], F32, tag="scsb", name="sc_sb")
                        nc.scalar.activation(sc_sb[:, :nb], sc_ps[:, :nb],
                                             AF.Identity, bias=cs["color_b"][:])
                        nc.vector.tensor_mul(outT_sb[:, t0 : t0 + nb],
                                             sc_sb[:, :nb],
                                             pT_sb[:, t0 : t0 + nb])

                # software pipeline: a pair's aggregates/node update are
                # emitted after the next pair's h stages so PE/ACT never
                # stall on DVE reduces
                pg_tiles = [None]
                hin_pair = [None]
                hout_pair = [None]
                pend = {}
                for t in range(NT + skew):
                    if t < NT:
                        ph = stage_h(t)
                        if t % 2 == 1:
                            pend[t - 1] = ph
                    tp = t - skew
                    if tp >= 1 and tp % 2 == 1:
                        stage_node(tp - 1, pend.pop(tp - 1))
                if last:
                    nc.sync.dma_start(out=out_d[:], in_=outT_sb[:])

                # ---- layer tail: color update (l < 2) ----
                if not last:
                    nc.vector.tensor_add(csum_acc[:], csum_acc[:], csum_acc2[:])
                    nc.sync.dma_start(out=cc_in[l][:], in_=csum_acc[:])
                    if sim_mode:
                        nc.sync.dma_start(out=cc_out[l][:], in_=cc_in[l][:])
                    else:
                        nc.gpsimd.collective_compute(
                            "AllReduce", mybir.AluOpType.add,
                            replica_groups=[list(range(NCORES))],
                            ins=[cc_in[l][:].opt()], outs=[cc_out[l][:].opt()],
                        )
                    csg = smallp.tile([H, C], F32, tag="csg")
                    nc.sync.dma_start(out=csg[:], in_=cc_out[l][:])
                    csg_bf = smallp.tile([H, C], BF16, tag="csgbf")
                    nc.vector.tensor_copy(csg_bf[:], csg[:])
                    zc_ps = sps.tile([H, C], F32, name="smp", tag="sm")
                    nc.tensor.matmul(zc_ps[:], lhsT=cs[f"U1{l}"][:], rhs=xcT[:],
                                     start=True, stop=False)
                    nc.tensor.matmul(zc_ps[:], lhsT=cs[f"W2U2{l}"][:], rhs=csg_bf[:],
                                     start=False, stop=True)
                    sc2 = smallp.tile([H, C], BF16, tag="sc2")
                    nc.scalar.activation(sc2[:], zc_ps[:], AF.Relu,
                                         bias=cs[f"bc{l}"][:])
                    xc_ps = sps.tile([H, C], F32, name="smp", tag="sm")
                    nc.tensor.matmul(xc_ps[:], lhsT=cs[f"V{l}"][:], rhs=sc2[:],
                                     start=True, stop=True)
                    nc.scalar.activation(xcT[:], xc_ps[:], AF.Identity,
                                         bias=cs[f"nb2{l}"][:])

    _split_multi_waits(nc)
    return nc


def make_in_maps(inputs):
    c = _consts(inputs)
    probs = np.asarray(inputs["probs"], np.float32)
    in_maps = []
    for k in range(NCORES):
        sl = probs[k * NB : (k + 1) * NB]          # [NB, C]
        m = dict(c)
        m["pT"] = np.ascontiguousarray(sl.T)                     # [C, NB] f32
        m["pTbf"] = np.ascontiguousarray(sl.T).astype(ml_dtypes.bfloat16)
        m["pbf"] = sl.reshape(1, -1).astype(ml_dtypes.bfloat16)  # [1, NB*C]
        in_maps.append(m)
    return in_maps


_NC_CACHE = None


def kernel(**inputs) -> np.ndarray:
    global _NC_CACHE
    _patch_tile_drain()
    if _NC_CACHE is None:
        _NC_CACHE = build_nc(skew=2, l2_dve_relu=True, hin_bufs=2, hout_bufs=2)
    nc = _NC_CACHE
    in_maps = make_in_maps(inputs)
    res = run_bass_kernel_spmd(nc, in_maps, core_ids=list(range(NCORES)))
    outT = np.concatenate([res.results[k]["outT"] for k in range(NCORES)], axis=1)
    return np.ascontiguousarray(outT.T).astype(np.float32)


# revision 27
# speedup vs baseline: 1.7444x; 1.7444x over previous
"""ColorGNN Trainium2 kernel: 3-layer message passing on the complete bipartite
graph (50000 birds x 16 colors, H=128), sharded by birds across 8 NeuronCores.

Reformulation (validated vs reference to ~6e-3 rel in bf16):
  Split eW1[l] into W1a (bird-x part), W1b (color-x part), W1c (edge-attr part).
  Track h^l = relu(edge_in @ eW1[l] + eb1[l]) instead of edge_attr: then
    h^0      = relu(p * u0 + A0[i] + B0[c] + c0),   u0 = edge_W @ W1c0
    h^l      = relu(h^{l-1} @ Wf_l + A_l[i] + B_l[c] + c_l),  Wf_l = W2_{l-1} @ W1c_l
    aggr     = (sum h) @ W2_l + deg*eb2_l   (aggregation commutes with the linear W2)
  so the only per-edge matmuls are the layer transitions, and the scatter-adds
  become per-bird sums (local, 16 edges) + per-color sums (AllReduce of [H,16]).

Layout: per-tile h kept H-major [H=128 partitions, edges free], bird-major edge
order (e = i*16 + c), 112 birds per tile so the per-edge adds A[i]+B[c] are ONE
extra accumulating matmul with a constant two-hot rhs (rows 0..111 birds,
112..127 colors).
"""

import numpy as np
import ml_dtypes

import concourse.bass as bass
import concourse.mybir as mybir
import concourse.tile as tile
from concourse.bass_utils import run_bass_kernel_spmd

F32 = mybir.dt.float32
BF16 = mybir.dt.bfloat16
AF = mybir.ActivationFunctionType

NCORES = 8
N, C, H, L = 50000, 16, 128, 3
NB = N // NCORES            # 6250 birds per core
TB = 112                    # birds per tile
NT = (NB + TB - 1) // TB    # 56 tiles (last has 90 birds)
NE = TB * C                 # 1792 edge columns per full tile
HALF = 896                  # h psum half-tile (2 PSUM banks)


def _patch_tile_drain():
    """Kept for API compat; wait-splitting now happens in _split_multi_waits."""


def _split_multi_waits(nc):
    """walrus in this env allows only ONE sync-wait per instruction. For any
    instruction with more waits, hoist the extras onto same-engine nops
    inserted immediately before it (sequencers execute in program order)."""
    k = 0
    for f in nc.m.functions:
        for blk in f.blocks:
            insts = blk.instructions
            out = []
            for inst in insts:
                si = inst.sync_info
                if si is not None and si.on_wait and len(si.on_wait) > 1:
                    waits = list(si.on_wait)
                    for w in waits[:-1]:
                        nop = mybir.InstNoOp(
                            name=f"waitnop-{k}", engine=inst.engine
                        )
                        k += 1
                        nop.sync_info = mybir.SyncInfo(on_wait=[w], on_update=[])
                        out.append(nop)
                    si.on_wait = waits[-1:]
                out.append(inst)
            if len(out) != len(insts):
                blk.instructions = out


def _bf(a):
    return np.ascontiguousarray(np.asarray(a, np.float64)).astype(ml_dtypes.bfloat16)


def _f32(a):
    return np.ascontiguousarray(np.asarray(a, np.float64).astype(np.float32))


def _consts(inp):
    """Host-side weight folding. Returns dict of name -> np array (replicated)."""
    f = {k: np.asarray(v, np.float64) for k, v in inp.items()}
    eW1, eb1, eW2, eb2 = f["eW1"], f["eb1"], f["eW2"], f["eb2"]
    nW1, nb1, nW2, nb2 = f["nW1"], f["nb1"], f["nW2"], f["nb2"]
    W1a = [eW1[l][:H] for l in range(L)]
    W1b = [eW1[l][H : 2 * H] for l in range(L)]
    W1c = [eW1[l][2 * H :] for l in range(L)]
    W2 = [eW2[l] for l in range(L)]
    U1 = [nW1[l][:H] for l in range(L)]
    U2 = [nW1[l][H:] for l in range(L)]

    c = {}
    c["u0"] = _bf(f["edge_W"][0] @ W1c[0]).reshape(1, H)  # lhsT [1, H]
    cvec = [
        f["edge_b"] @ W1c[0] + eb1[0],
        eb2[0] @ W1c[1] + eb1[1],
        eb2[1] @ W1c[2] + eb1[2],
    ]
    wmats, vvecs = {}, {}
    for l in range(L):
        vvecs[f"cvec{l}"] = cvec[l]
        wmats[f"W1a{l}"] = W1a[l]               # rhs [H, H]
        wmats[f"W1b{l}"] = W1b[l]               # rhs [H, H]
        wmats[f"U1{l}"] = U1[l]                 # lhsT [H, H]
        wmats[f"W2U2{l}"] = W2[l] @ U2[l]       # lhsT [H, H]
        wmats[f"V{l}"] = nW2[l]                 # lhsT [H, H]
        vvecs[f"bb{l}"] = nb1[l] + C * (eb2[l] @ U2[l])
        vvecs[f"nb2{l}"] = nb2[l]
        if l < L - 1:
            vvecs[f"bc{l}"] = nb1[l] + N * (eb2[l] @ U2[l])
    wmats["Wf1"] = W2[0] @ W1c[1]               # lhsT [H, H]
    wmats["Wf2"] = W2[1] @ W1c[2]               # lhsT [H, H]
    vvecs["node_b"] = f["node_b"]
    # layer-0 folds: x0 = p @ node_W + node_b never materialized on device
    vvecs["cvec0"] = vvecs["cvec0"] + f["node_b"] @ W1a[0]
    vvecs["bb0"] = vvecs["bb0"] + f["node_b"] @ U1[0]
    c["NW1a"] = _bf(f["node_W"] @ W1a[0])       # [C, H]
    c["NWU1"] = _bf(f["node_W"] @ U1[0])        # [C, H]
    c["wpack"] = _bf(np.concatenate([wmats[k] for k in sorted(wmats)], axis=1))
    c["vpack"] = _f32(np.stack([vvecs[k] for k in sorted(vvecs)], axis=1))
    c["node_W"] = _bf(f["node_W"])              # lhsT [C, H]
    c["xc0T"] = _bf((f["node_W"] + f["node_b"]).T)  # [H, C]
    c["color_W"] = _bf(f["color_W"])            # lhsT [H, C]
    c["color_b"] = _f32(f["color_b"]).reshape(C, 1)
    # two-hot matrix for the A/B add: col e -> 1 at row e//16 and row 112+e%16
    oh = np.zeros((H, NE), np.float32)
    e = np.arange(NE)
    oh[e // C, e] = 1.0
    oh[TB + (e % C), e] = 1.0
    c["twohot"] = oh.astype(ml_dtypes.bfloat16)
    return c


def build_nc(sim_mode=False, skew=1, hin_bufs=3, hout_bufs=3, ab_bufs=3,
             hps_bufs=2, sps_bufs=4, l2_dve_relu=False):
    nc = bass.Bass(num_devices=1 if sim_mode else NCORES)

    wnames = sorted(
        [f"{nm}{l}" for l in range(L) for nm in ("W1a", "W1b", "U1", "W2U2", "V")]
        + ["Wf1", "Wf2"]
    )
    vnames = sorted(
        [f"cvec{l}" for l in range(L)] + [f"bb{l}" for l in range(L)]
        + [f"nb2{l}" for l in range(L)] + ["bc0", "bc1", "node_b"]
    )
    cshapes = {
        "u0": ([1, H], BF16), "node_W": ([C, H], BF16),
        "xc0T": ([H, C], BF16), "color_W": ([H, C], BF16), "color_b": ([C, 1], F32),
        "twohot": ([H, NE], BF16),
        "NW1a": ([C, H], BF16), "NWU1": ([C, H], BF16),
        "wpack": ([H, len(wnames) * H], BF16),
        "vpack": ([H, len(vnames)], F32),
    }

    dins = {}
    for name, (shp, dt) in cshapes.items():
        dins[name] = nc.declare_dram_parameter(name, shp, dt, isOutput=False)
    dins["pT"] = nc.declare_dram_parameter("pT", [C, NB], F32, isOutput=False)
    dins["pTbf"] = nc.declare_dram_parameter("pTbf", [C, NB], BF16, isOutput=False)
    dins["pbf"] = nc.declare_dram_parameter("pbf", [1, NB * C], BF16, isOutput=False)
    out_d = nc.declare_dram_parameter("outT", [C, NB], F32, isOutput=True)

    with tile.TileContext(nc) as tc:
        with (
            tc.tile_pool(name="const", bufs=1) as constp,
            tc.tile_pool(name="xpool", bufs=1) as xpool,
            tc.tile_pool(name="hsb", bufs=3) as hsbp,
            tc.tile_pool(name="ab", bufs=ab_bufs) as abp,
            tc.tile_pool(name="ptile", bufs=2) as ptp,
            tc.tile_pool(name="agg", bufs=4) as aggp,
            tc.tile_pool(name="small", bufs=4) as smallp,
            tc.tile_pool(name="hps", bufs=hps_bufs, space="PSUM") as hps,
            tc.tile_pool(name="sps", bufs=sps_bufs, space="PSUM") as sps,
            tc.tile_pool(name="dram", bufs=1, space="DRAM") as dramp,
        ):
            # ---- load constants ----
            cs = {}
            for name, (shp, dt) in cshapes.items():
                cs[name] = constp.tile(shp, dt, name=f"c_{name}")
                nc.sync.dma_start(out=cs[name][:], in_=dins[name][:])
            for i, nm in enumerate(wnames):
                cs[nm] = cs["wpack"][:, i * H : (i + 1) * H]
            for i, nm in enumerate(vnames):
                cs[nm] = cs["vpack"][:, i : i + 1]
            # p data resident in SBUF (cuts per-tile DMA count)
            pTbf_sb = constp.tile([C, NB], BF16, name="pTbf_sb")
            nc.sync.dma_start(out=pTbf_sb[:], in_=dins["pTbf"][:])
            pT_sb = constp.tile([C, NB], F32, name="pT_sb")
            nc.sync.dma_start(out=pT_sb[:], in_=dins["pT"][:])
            outT_sb = constp.tile([C, NB], F32, name="outT_sb")

            xT = xpool.tile([H, NT * TB], BF16)       # bird states, H-major
            xcT = xpool.tile([H, C], BF16)            # color states
            nc.sync.dma_start(out=xcT[:], in_=dins["xc0T"][:])
            csum_acc = xpool.tile([H, C], F32)
            csum_acc2 = xpool.tile([H, C], F32)

            H0R = 24   # h0 tiles resident in SBUF (no DRAM round-trip)
            h0_sb = constp.tile([H, H0R * NE], BF16, name="h0_sb")
            h_d = [dramp.tile([H, NT * NE], BF16, name=f"h_d{i}") for i in range(2)]
            AB_SLOTS = 4
            ab_tiles = [xpool.tile([H, H], BF16, name=f"abslot{i}")
                        for i in range(AB_SLOTS)]
            cc_in = [dramp.tile([H, C], F32, name=f"cc_in{i}") for i in range(2)]
            cc_out = [dramp.tile([H, C], F32, name=f"cc_out{i}") for i in range(2)]

            def tcols(t):
                nb = min(TB, NB - t * TB)
                return t * TB, nb

            # ================= pass l =================
            for l in range(L):
                last = l == L - 1
                # B_l [C, H] bird-major for the two-hot lhsT rows 112:128
                src = cs["xc0T"] if l == 0 else xcT
                b_ps = sps.tile([C, H], F32, name="smp", tag="sm")
                nc.tensor.matmul(b_ps[:], lhsT=src[:], rhs=cs[f"W1b{l}"][:],
                                 start=True, stop=True)
                B_sb = smallp.tile([C, H], BF16)
                nc.scalar.copy(B_sb[:], b_ps[:])
                for abt in ab_tiles:
                    nc.sync.dma_start(out=abt[TB:, :], in_=B_sb[:])
                if not last:
                    nc.vector.memset(csum_acc[:], 0.0)
                    nc.vector.memset(csum_acc2[:], 0.0)

                def stage_h(t):
                    """DMA in, build AB, h matmuls + relu, store h, reduces."""
                    t0, nb = tcols(t)
                    ne = nb * C
                    ec0 = t * NE

                    prow = hin = None
                    if l == 0:
                        if t % 2 == 0:
                            pg_tiles[0] = ptp.tile([1, 2 * NE], BF16,
                                                   tag="prow", name="prow")
                            c0 = t * TB * C
                            cn = min(2 * NE, NB * C - c0)
                            nc.sync.dma_start(out=pg_tiles[0][:, :cn],
                                              in_=dins["pbf"][:, c0 : c0 + cn])
                        prow = pg_tiles[0][:, (t % 2) * NE : (t % 2) * NE + ne]
                    elif l == 1 and t < H0R:
                        hin = h0_sb[:, t * NE : t * NE + NE]
                    else:
                        hin = hsbp.tile([H, NE], BF16, tag="hin", name="hin",
                                        bufs=hin_bufs)
                        nc.sync.dma_start(out=hin[:, :ne],
                                          in_=h_d[l - 1][:, ec0 : ec0 + ne])

                    # A_l = x^l @ W1a_l, bird-major [nb, H]
                    # (l=0: A0 = p @ (node_W @ W1a0), folded on host)
                    a_ps = sps.tile([TB, H], F32, name="smp", tag="sm")
                    if l == 0:
                        nc.tensor.matmul(a_ps[:nb, :],
                                         lhsT=pTbf_sb[:, t0 : t0 + nb],
                                         rhs=cs["NW1a"][:], start=True, stop=True)
                    else:
                        nc.tensor.matmul(a_ps[:nb, :], lhsT=xT[:, t0 : t0 + nb],
                                         rhs=cs[f"W1a{l}"][:], start=True, stop=True)
                    ab = ab_tiles[t % AB_SLOTS]
                    if nb < TB:
                        nc.vector.memset(ab[:TB, :], 0.0)
                    nc.vector.tensor_copy(ab[:nb, :], a_ps[:nb, :])

                    # ---- h tile: psum halves + relu ----
                    if l == 0 and t < H0R:
                        h_sb = h0_sb[:, t * NE : t * NE + NE]
                    else:
                        h_sb = hsbp.tile([H, NE], BF16, tag="hout", name="h_sb",
                                         bufs=hout_bufs)
                    for cbase in range(0, ne, HALF):
                        cw = min(HALF, ne - cbase)
                        ps = hps.tile([H, HALF], F32, name="hps")
                        for q0 in range(0, cw, 512):
                            qw = min(512, cw - q0)
                            sl = slice(cbase + q0, cbase + q0 + qw)
                            if l == 0:
                                nc.tensor.matmul(ps[:, q0 : q0 + qw],
                                                 lhsT=cs["u0"][:],
                                                 rhs=prow[:, sl],
                                                 start=True, stop=False)
                            else:
                                nc.tensor.matmul(ps[:, q0 : q0 + qw],
                                                 lhsT=cs[f"Wf{l}"][:],
                                                 rhs=hin[:, sl],
                                                 start=True, stop=False)
                            nc.tensor.matmul(ps[:, q0 : q0 + qw], lhsT=ab[:],
                                             rhs=cs["twohot"][:, sl],
                                             start=False, stop=True)
                        if l2_dve_relu and last and cbase > 0:
                            nc.vector.tensor_scalar(
                                h_sb[:, cbase : cbase + cw], ps[:, :cw],
                                cs[f"cvec{l}"][:], 0.0,
                                op0=mybir.AluOpType.add,
                                op1=mybir.AluOpType.max)
                        else:
                            nc.scalar.activation(h_sb[:, cbase : cbase + cw],
                                                 ps[:, :cw], AF.Relu,
                                                 bias=cs[f"cvec{l}"][:])

                    if not last and not (l == 0 and t < H0R):
                        nc.sync.dma_start(out=h_d[l][:, ec0 : ec0 + ne],
                                          in_=h_sb[:, :ne])
                    if not last:
                        h3 = h_sb[:, :ne].rearrange("p (b c) -> p b c", c=C)
                        csum_t = aggp.tile([H, C], F32, tag="csumt", name="csum_t")
                        nc.vector.tensor_reduce(csum_t[:], h3.transpose([0, 2, 1]),
                                                axis=mybir.AxisListType.X,
                                                op=mybir.AluOpType.add)
                        acc = csum_acc if t % 2 == 0 else csum_acc2
                        nc.vector.tensor_add(acc[:], acc[:], csum_t[:])
                    return h_sb

                def stage_node(t, h_sb):
                    """Bird node update for tile t (+ scores on the last pass).
                    The bird-sum over each bird's 16 edges rides the z matmul
                    group as 16 strided accumulating matmuls (fuses the W2@U2
                    projection of the aggregate for free)."""
                    t0, nb = tcols(t)
                    h3 = h_sb[:, : nb * C].rearrange("p (b c) -> p b c", c=C)
                    z_ps = sps.tile([H, TB], F32, name="smp", tag="sm")
                    if l == 0:
                        nc.tensor.matmul(z_ps[:, :nb], lhsT=cs["NWU1"][:],
                                         rhs=pTbf_sb[:, t0 : t0 + nb],
                                         start=True, stop=False)
                    else:
                        nc.tensor.matmul(z_ps[:, :nb], lhsT=cs[f"U1{l}"][:],
                                         rhs=xT[:, t0 : t0 + nb],
                                         start=True, stop=False)
                    for cc in range(C):
                        nc.tensor.matmul(z_ps[:, :nb], lhsT=cs[f"W2U2{l}"][:],
                                         rhs=h3[:, :, cc], start=False,
                                         stop=(cc == C - 1))
                    s_sb = smallp.tile([H, TB], BF16, tag="ssb", name="s_sb")
                    nc.scalar.activation(s_sb[:, :nb], z_ps[:, :nb], AF.Relu,
                                         bias=cs[f"bb{l}"][:])
                    x_ps = sps.tile([H, TB], F32, name="smp", tag="sm")
                    nc.tensor.matmul(x_ps[:, :nb], lhsT=cs[f"V{l}"][:],
                                     rhs=s_sb[:, :nb], start=True, stop=True)
                    nc.scalar.activation(xT[:, t0 : t0 + nb], x_ps[:, :nb],
                                         AF.Identity, bias=cs[f"nb2{l}"][:])

                    if last:
                        # scores_T = color_W.T @ x3 + color_b ; out = scores * p
                        sc_ps = sps.tile([C, TB], F32, name="smp", tag="sm")
                        nc.tensor.matmul(sc_ps[:, :nb], lhsT=cs["color_W"][:],
                                         rhs=xT[:, t0 : t0 + nb],
                                         start=True, stop=True)
                        sc_sb = smallp.tile([C, TB], F32, tag="scsb", name="sc_sb")
                        nc.scalar.activation(sc_sb[:, :nb], sc_ps[:, :nb],
                                             AF.Identity, bias=cs["color_b"][:])
                        nc.vector.tensor_mul(outT_sb[:, t0 : t0 + nb],
                                             sc_sb[:, :nb],
                                             pT_sb[:, t0 : t0 + nb])

                # software pipeline: tile t's node update is emitted after
                # tile t+1's h stage so PE/ACT never stall on DVE reduces
                pg_tiles = [None]
                pend = {}
                for t in range(NT + skew):
                    if t < NT:
                        pend[t] = stage_h(t)
                    if t >= skew:
                        stage_node(t - skew, pend.pop(t - skew))
                if last:
                    nc.sync.dma_start(out=out_d[:], in_=outT_sb[:])

                # ---- layer tail: color update (l < 2) ----
                if not last:
                    nc.vector.tensor_add(csum_acc[:], csum_acc[:], csum_acc2[:])
                    nc.sync.dma_start(out=cc_in[l][:], in_=csum_acc[:])
                    if sim_mode:
                        nc.sync.dma_start(out=cc_out[l][:], in_=cc_in[l][:])
                    else:
                        nc.gpsimd.collective_compute(
                            "AllReduce", mybir.AluOpType.add,
                            replica_groups=[list(range(NCORES))],
                            ins=[cc_in[l][:].opt()], outs=[cc_out[l][:].opt()],
                        )
                    csg = smallp.tile([H, C], F32, tag="csg")
                    nc.sync.dma_start(out=csg[:], in_=cc_out[l][:])
                    csg_bf = smallp.tile([H, C], BF16, tag="csgbf")
                    nc.vector.tensor_copy(csg_bf[:], csg[:])
                    zc_ps = sps.tile([H, C], F32, name="smp", tag="sm")
                    nc.tensor.matmul(zc_ps[:], lhsT=cs[f"U1{l}"][:], rhs=xcT[:],
                                     start=True, stop=False)
                    nc.tensor.matmul(zc_ps[:], lhsT=cs[f"W2U2{l}"][:], rhs=csg_bf[:],
                                     start=False, stop=True)
                    sc2 = smallp.tile([H, C], BF16, tag="sc2")
                    nc.scalar.activation(sc2[:], zc_ps[:], AF.Relu,
                                         bias=cs[f"bc{l}"][:])
                    xc_ps = sps.tile([H, C], F32, name="smp", tag="sm")
                    nc.tensor.matmul(xc_ps[:], lhsT=cs[f"V{l}"][:], rhs=sc2[:],
                                     start=True, stop=True)
                    nc.scalar.activation(xcT[:], xc_ps[:], AF.Identity,
                                         bias=cs[f"nb2{l}"][:])

    _split_multi_waits(nc)
    return nc


def make_in_maps(inputs):
    c = _consts(inputs)
    probs = np.asarray(inputs["probs"], np.float32)
    in_maps = []
    for k in range(NCORES):
        sl = probs[k * NB : (k + 1) * NB]          # [NB, C]
        m = dict(c)
        m["pT"] = np.ascontiguousarray(sl.T)                     # [C, NB] f32
        m["pTbf"] = np.ascontiguousarray(sl.T).astype(ml_dtypes.bfloat16)
        m["pbf"] = sl.reshape(1, -1).astype(ml_dtypes.bfloat16)  # [1, NB*C]
        in_maps.append(m)
    return in_maps


_NC_CACHE = None


def kernel(**inputs) -> np.ndarray:
    global _NC_CACHE
    _patch_tile_drain()
    if _NC_CACHE is None:
        _NC_CACHE = build_nc(skew=2, l2_dve_relu=True)
    nc = _NC_CACHE
    in_maps = make_in_maps(inputs)
    res = run_bass_kernel_spmd(nc, in_maps, core_ids=list(range(NCORES)))
    outT = np.concatenate([res.results[k]["outT"] for k in range(NCORES)], axis=1)
    return np.ascontiguousarray(outT.T).astype(np.float32)
